# revision 1
# baseline (speedup 1.0000x reference)
"""Trainium2 Bass kernel for nn_Merge_Model (gnn_message_passing).

Self-contained: takes FULL inputs as numpy arrays, shards across 8
NeuronCores internally, runs one SPMD Bass program, gathers the output.

Sharding:
  - word graph: edges + segment-mean sharded by dst-node range (NW/8 per core)
  - x_batch mean-pool: resharded by word range (count-matrix matmul) with a
    ReduceScatter of the [B, D] partial sums
  - doc graph / attention / fc: data-parallel over docs (B/8 per core)

Device-side algorithm notes:
  - (seq @ W + b).mean(L) == mean_L(seq) @ W + b  (linearity)
  - seg-mean via PE: lhsT = 0/1 selection tile built with DVE is_equal
    against an iota row; messages fetched with dma_gather using host-fused
    indices src_nid[e_src] from a 320-float-padded copy of emb_word
    (256B-stride requirement), split into low/high halves (int16 indices)
  - all big matmuls in float32r (1 cycle/row when N >= 256)
"""

import math
import os
import sys
from contextlib import ExitStack

import numpy as np

for _p in ("/opt/trn_rl_repo", "/root/.axon_site/_ro/trn_rl_repo"):
    if os.path.isdir(_p) and _p not in sys.path:
        sys.path.insert(0, _p)

import concourse.bass as bass
import concourse.bacc as bacc
import concourse.mybir as mybir
import concourse.tile as tile
from concourse.bass import IndirectOffsetOnAxis
from concourse.bass_utils import run_bass_kernel_spmd
from concourse.masks import make_identity

F32 = mybir.dt.float32
F32R = mybir.dt.float32r
I32 = mybir.dt.int32
I16 = mybir.dt.int16
ADD = mybir.AluOpType.add
MUL = mybir.AluOpType.mult
EQ = mybir.AluOpType.is_equal
Exp = mybir.ActivationFunctionType.Exp
Ident = mybir.ActivationFunctionType.Identity
AxX = mybir.AxisListType.X

NCORES = 8
P = 128          # partitions
DC = 100         # contraction chunk of D
DP = 320         # padded embedding row (1280 B, multiple of 256)
GT = 8           # message-gather granularity (tiles per dma_gather call)
HALF = 32768     # int16 index range per table half


def _wrap16(flat):
    """dma_gather index layout: idx j at [j%16, j//16] (pre-replication)."""
    n = len(flat)
    assert n % 128 == 0
    return np.asarray(flat, np.int16).reshape(n // 16, 16).T   # [16, n/16]


# --------------------------------------------------------------------------
# host-side integer/index preprocessing (sharding prep; no float math)
# --------------------------------------------------------------------------

def _shard_word_edges(src_nid, e_src, e_dst, nd, nb):
    gidx_all = src_nid[e_src]
    order = np.argsort(e_dst, kind="stable")
    ds = e_dst[order]
    gs = gidx_all[order]
    per_core = []
    for c in range(NCORES):
        lo, hi = np.searchsorted(ds, [c * nd, (c + 1) * nd])
        dloc = ds[lo:hi] - c * nd
        gloc = gs[lo:hi]
        blk = dloc // P
        groups = []
        for b in range(nb):
            blo, bhi = np.searchsorted(blk, [b, b + 1])
            g = gloc[blo:bhi]
            col = dloc[blo:bhi] - b * P
            m = g < HALF
            groups.append(((g[m], col[m]), (g[~m] - HALF, col[~m])))
        per_core.append(groups)
    return per_core


def prep(inputs):
    ew = inputs["emb_word"]
    VW, D = ew.shape
    NW = inputs["dst_nid"].shape[0]
    B, L = inputs["x_batch"].shape
    SD = inputs["src_nid_dis_d"].shape[0]
    C = inputs["w_fc"].shape[1]
    assert D % DC == 0 and NW % NCORES == 0 and B % NCORES == 0
    nd = NW // NCORES
    nb = (nd + P - 1) // P
    bl = B // NCORES
    nzc = SD // P
    NCH = D // DC

    rels = [
        (inputs["src_nid_dis"], inputs["e_src_dis"], inputs["e_dst_dis"]),
        (inputs["src_nid_pmi"], inputs["e_src_pmi"], inputs["e_dst_pmi"]),
        (inputs["src_nid_top"], inputs["e_src_top"], inputs["e_dst_top"]),
    ]
    sharded = [
        _shard_word_edges(sn.astype(np.int64), es.astype(np.int64),
                          edst.astype(np.int64), nd, nb)
        for (sn, es, edst) in rels
    ]

    # fixed tile counts per (block, rel, half): max over cores
    tfix = np.zeros((nb, 3, 2), dtype=np.int64)
    for r in range(3):
        for c in range(NCORES):
            for b in range(nb):
                for h in range(2):
                    n = len(sharded[r][c][b][h][0])
                    tfix[b, r, h] = max(tfix[b, r, h], (n + P - 1) // P)
    for b in range(nb):
        for r in range(3):
            if tfix[b, r].sum() == 0:
                tfix[b, r, 0] = 1
    TT = int(tfix.sum())

    # stream segments in (block, rel, half) order
    stream = []      # (b, r, h, ntiles)
    for b in range(nb):
        for r in range(3):
            for h in range(2):
                if tfix[b, r, h] > 0:
                    stream.append((b, r, int(h), int(tfix[b, r, h])))

    core_inputs = []
    for c in range(NCORES):
        gidx_flat = np.zeros((TT * P,), dtype=np.int64)
        col_flat = np.full((TT * P,), -1.0, dtype=np.float32)
        pos = 0
        for (b, r, h, nt) in stream:
            g, col = sharded[r][c][b][h]
            n = len(g)
            room = nt * P
            assert n <= room
            gidx_flat[pos:pos + n] = g
            col_flat[pos:pos + n] = col.astype(np.float32)
            pos += room
        assert pos == TT * P
        ix16 = np.tile(_wrap16(gidx_flat), (8, 1))           # [128, TT*8]
        scol = np.ascontiguousarray(col_flat.reshape(TT, P).T)  # [128, TT]

        rcnt = np.ones((P, nb * 3), dtype=np.float32)
        for r in range(3):
            cnt = np.zeros((nb * P,), dtype=np.int64)
            for b_i in range(nb):
                (g0, c0), (g1, c1) = sharded[r][c][b_i]
                col = np.concatenate([c0, c1])
                if len(col):
                    cnt[b_i * P:(b_i + 1) * P] = np.bincount(
                        col.astype(np.int64), minlength=P)
            rec = 1.0 / np.maximum(cnt, 1).astype(np.float32)
            rcnt[:, [b_i * 3 + r for b_i in range(nb)]] = rec.reshape(nb, P).T

        dloc = inputs["dst_nid"][c * nd:(c + 1) * nd].astype(np.int64)
        h0pad = np.zeros((nb * P,), dtype=np.int64)
        h0pad[:nd] = dloc
        h0_idx = np.ascontiguousarray(
            h0pad.reshape(nb, P).T).astype(np.int32)         # [128, nb]

        x = inputs["x_batch"].astype(np.int64).ravel()
        docs = np.repeat(np.arange(B, dtype=np.int64), L)
        m = (x >= c * nd) & (x < (c + 1) * nd)
        cm = np.bincount((x[m] - c * nd) * B + docs[m],
                         minlength=nd * B).astype(np.float32).reshape(nd, B)
        cmp_ = np.zeros((nb * P, B), dtype=np.float32)
        cmp_[:nd] = cm
        cmat = np.ascontiguousarray(
            cmp_.reshape(nb, P, B).transpose(1, 0, 2))       # [128, nb, B]

        zi = np.zeros((3 * nzc * P,), dtype=np.int64)
        cd = np.zeros((P, 3 * nzc, bl), dtype=np.float32)
        rcd = np.ones((bl, 3), dtype=np.float32)
        for r, (sn_d, es_d, ed_d) in enumerate([
            (inputs["src_nid_dis_d"], inputs["e_src_dis_d"], inputs["e_dst_dis_d"]),
            (inputs["src_nid_pmi_d"], inputs["e_src_pmi_d"], inputs["e_dst_pmi_d"]),
            (inputs["src_nid_top_d"], inputs["e_src_top_d"], inputs["e_dst_top_d"]),
        ]):
            zi[r * nzc * P:(r + 1) * nzc * P] = sn_d.astype(np.int64)
            es64 = es_d.astype(np.int64)
            ed64 = ed_d.astype(np.int64)
            md = (ed64 >= c * bl) & (ed64 < (c + 1) * bl)
            cdm = np.bincount(es64[md] * bl + (ed64[md] - c * bl),
                              minlength=SD * bl).astype(np.float32).reshape(SD, bl)
            cd[:, r * nzc:(r + 1) * nzc, :] = \
                cdm.reshape(nzc, P, bl).transpose(1, 0, 2)
            cntd = np.bincount(ed64[md] - c * bl, minlength=bl)
            rcd[:, r] = 1.0 / np.maximum(cntd, 1).astype(np.float32)
        zidx16 = np.tile(_wrap16(zi), (8, 1))                # [128, 3*nzc*8]

        core_inputs.append(dict(
            msg_ix=np.ascontiguousarray(ix16), scol=scol, rcnt=rcnt,
            h0_idx=h0_idx,
            cmat=np.ascontiguousarray(cmat.reshape(P, nb * B)),
            zidx=np.ascontiguousarray(zidx16),
            cd=np.ascontiguousarray(cd.reshape(P, 3 * nzc * bl)),
            rcd=rcd,
        ))

    def wstack(ws):
        arrs = [w.reshape(2, NCH, DC, w.shape[1]) for w in ws]
        cat = np.concatenate(arrs, axis=0).reshape(-1, NCH, DC, ws[0].shape[1])
        return np.ascontiguousarray(cat.transpose(2, 0, 1, 3)
                                    .reshape(DC, -1, ws[0].shape[1]))

    wW = wstack([inputs["w_dis"], inputs["w_pmi"], inputs["w_top"]])
    wDoc = wstack([inputs["w_dis_d"], inputs["w_pmi_d"], inputs["w_top_d"]])
    wdense = np.ascontiguousarray(
        inputs["w_dense"].reshape(NCH, DC, D).transpose(1, 0, 2))
    wfc = np.ascontiguousarray(
        inputs["w_fc"].reshape(NCH, DC, C).transpose(1, 0, 2))
    bias = np.zeros((1, 8, D), dtype=np.float32)
    for i, k in enumerate(("b_dis", "b_pmi", "b_top", "b_dis_d", "b_pmi_d",
                           "b_top_d", "b_dense")):
        bias[0, i, :] = inputs[k]
    bias[0, 7, :C] = inputs["b_fc"]

    # padded embedding tables (layout-only change; zero-filled tail)
    ewp = np.zeros((VW, DP), dtype=np.float32)
    ewp[:, :D] = inputs["emb_word"]
    edp = np.zeros((inputs["emb_doc"].shape[0], DP), dtype=np.float32)
    edp[:, :D] = inputs["emb_doc"]

    meta = dict(VW=VW, VD=edp.shape[0], D=D, NW=NW, B=B, L=L, SD=SD, C=C,
                nd=nd, nb=nb, bl=bl, nzc=nzc, NCH=NCH, TT=TT, stream=stream)

    in_maps = []
    for c in range(NCORES):
        m = dict(core_inputs[c])
        m["ewp"] = ewp
        m["edp"] = edp
        m["wW"] = wW.reshape(DC, -1)
        m["wDoc"] = wDoc.reshape(DC, -1)
        m["wdense"] = wdense.reshape(DC, -1)
        m["wfc"] = wfc.reshape(DC, -1)
        m["bias"] = bias.reshape(1, -1)
        in_maps.append(m)
    return meta, in_maps


# --------------------------------------------------------------------------
# device program
# --------------------------------------------------------------------------

def build_program(meta):
    VW, VD, D, B, C = meta["VW"], meta["VD"], meta["D"], meta["B"], meta["C"]
    nb, bl, nzc, NCH, TT = (meta["nb"], meta["bl"], meta["nzc"],
                            meta["NCH"], meta["TT"])
    stream = meta["stream"]
    L = meta["L"]
    sqd = 1.0 / math.sqrt(D)

    nc = bacc.Bacc("TRN2", target_bir_lowering=False, debug=False)
    ewp = nc.declare_dram_parameter("ewp", [VW, DP], F32R, isOutput=False)
    edp = nc.declare_dram_parameter("edp", [VD, DP], F32R, isOutput=False)
    msg_ix_d = nc.declare_dram_parameter("msg_ix", [P, TT * 8], I16, isOutput=False)
    scol_d = nc.declare_dram_parameter("scol", [P, TT], F32, isOutput=False)
    rcnt_d = nc.declare_dram_parameter("rcnt", [P, nb * 3], F32, isOutput=False)
    h0_idx_d = nc.declare_dram_parameter("h0_idx", [P, nb], I32, isOutput=False)
    cmat_d = nc.declare_dram_parameter("cmat", [P, nb * B], F32, isOutput=False)
    zidx_d = nc.declare_dram_parameter("zidx", [P, 3 * nzc * 8], I16, isOutput=False)
    cd_d = nc.declare_dram_parameter("cd", [P, 3 * nzc * bl], F32, isOutput=False)
    rcd_d = nc.declare_dram_parameter("rcd", [bl, 3], F32, isOutput=False)
    wW_d = nc.declare_dram_parameter("wW", [DC, 6 * NCH * D], F32, isOutput=False)
    wDoc_d = nc.declare_dram_parameter("wDoc", [DC, 6 * NCH * D], F32, isOutput=False)
    wdense_d = nc.declare_dram_parameter("wdense", [DC, NCH * D], F32, isOutput=False)
    wfc_d = nc.declare_dram_parameter("wfc", [DC, NCH * C], F32, isOutput=False)
    bias_d = nc.declare_dram_parameter("bias", [1, 8 * D], F32, isOutput=False)
    out_logits = nc.declare_dram_parameter("out_logits", [bl, C], F32, isOutput=True)
    out_docrep = nc.declare_dram_parameter("out_docrep", [bl, D], F32, isOutput=True)

    with ExitStack() as ctx:
        tc = ctx.enter_context(tile.TileContext(nc))
        const_p = ctx.enter_context(tc.tile_pool(name="const", bufs=1))
        idx_p = ctx.enter_context(tc.tile_pool(name="idx", bufs=1))
        w_p = ctx.enter_context(tc.tile_pool(name="wts", bufs=1))
        msg_p = ctx.enter_context(tc.tile_pool(name="msg", bufs=2))
        sb_p = ctx.enter_context(tc.tile_pool(name="sb", bufs=3))
        h0_p = ctx.enter_context(tc.tile_pool(name="h0", bufs=3))
        nei_p = ctx.enter_context(tc.tile_pool(name="nei", bufs=4))
        tp_p = ctx.enter_context(tc.tile_pool(name="tp", bufs=14))
        h_p = ctx.enter_context(tc.tile_pool(name="h", bufs=4))
        small_p = ctx.enter_context(tc.tile_pool(name="small", bufs=8))
        scr_p = ctx.enter_context(tc.tile_pool(name="scr", bufs=2))
        tbl_p = ctx.enter_context(tc.tile_pool(name="tbl", bufs=1))
        cm_p = ctx.enter_context(tc.tile_pool(name="cm", bufs=2))
        zd_p = ctx.enter_context(tc.tile_pool(name="zd", bufs=2))
        dram_p = ctx.enter_context(tc.tile_pool(name="dram", bufs=1, space="DRAM"))
        ps_seg = ctx.enter_context(tc.tile_pool(name="ps_seg", bufs=3, space="PSUM"))
        ps_mm = ctx.enter_context(tc.tile_pool(name="ps_mm", bufs=2, space="PSUM"))
        ps_tp = ctx.enter_context(tc.tile_pool(name="ps_tp", bufs=2, space="PSUM"))

        # ---- constants ----
        iota_i = const_p.tile([P, P], I32)
        nc.gpsimd.iota(iota_i[:], pattern=[[1, P]], base=0, channel_multiplier=0)
        iota_f = const_p.tile([P, P], F32)
        nc.vector.tensor_copy(out=iota_f[:], in_=iota_i[:])
        ident = const_p.tile([P, P], F32)
        make_identity(nc, ident[:])
        identS = const_p.tile([bl, bl], F32)
        make_identity(nc, identS[:])
        ones1f = const_p.tile([1, P], F32)
        nc.vector.memset(ones1f[:], 1.0)
        ones1 = const_p.tile([1, P], F32R)
        nc.vector.tensor_copy(out=ones1[:], in_=ones1f[:])

        # ---- static loads ----
        msg_ix = idx_p.tile([P, TT * 8], I16)
        nc.sync.dma_start(out=msg_ix[:], in_=msg_ix_d[:])
        scol = idx_p.tile([P, TT], F32)
        nc.sync.dma_start(out=scol[:], in_=scol_d[:])
        rcnt = idx_p.tile([P, nb * 3], F32)
        nc.sync.dma_start(out=rcnt[:], in_=rcnt_d[:])
        h0_idx = idx_p.tile([P, nb], I32)
        nc.sync.dma_start(out=h0_idx[:], in_=h0_idx_d[:])
        zidx = idx_p.tile([P, 3 * nzc * 8], I16)
        nc.sync.dma_start(out=zidx[:], in_=zidx_d[:])
        cdm = idx_p.tile([P, 3 * nzc * bl], F32R)
        nc.gpsimd.dma_start(out=cdm[:], in_=cd_d[:])
        rcd = idx_p.tile([bl, 3], F32)
        nc.sync.dma_start(out=rcd[:], in_=rcd_d[:])
        wW = w_p.tile([DC, 6 * NCH * D], F32R, tag="wbig")
        nc.gpsimd.dma_start(out=wW[:], in_=wW_d[:])
        wdense = w_p.tile([DC, NCH * D], F32R)
        nc.gpsimd.dma_start(out=wdense[:], in_=wdense_d[:])
        wfc = w_p.tile([DC, NCH * C], F32R)
        nc.gpsimd.dma_start(out=wfc[:], in_=wfc_d[:])
        bias = w_p.tile([1, 8 * D], F32R)
        nc.gpsimd.dma_start(out=bias[:], in_=bias_d[:])

        def bias_mm(psum_ap, i, parts, width=None):
            w = D if width is None else width
            nc.tensor.matmul(out=psum_ap, lhsT=ones1[:, :parts],
                             rhs=bias[:, i * D:i * D + w],
                             start=False, stop=True)

        tbl = tbl_p.tile([P, nb * D], F32R)
        KDBG = os.environ.get("K_DBG", "")
        dbg = const_p.tile([P, D], F32, name="dbg") if KDBG else None

        # ---- word-graph phase ----
        call_plan = []        # (t0, ntiles, half)
        cursor = 0
        for (b, r, h, nt) in stream:
            t = 0
            while t < nt:
                n = min(GT, nt - t)
                call_plan.append((cursor + t, n, h))
                t += n
            cursor += nt

        seg_of_tile = {}
        call_issued = [0]

        def ensure_calls(upto_tile):
            while call_issued[0] < len(call_plan):
                t0, ntc, h = call_plan[call_issued[0]]
                if t0 > upto_tile:
                    return
                mb = msg_p.tile([P, GT * DP], F32R, tag="msg", name="mb")
                if VW > HALF:
                    src = ewp[:HALF, :] if h == 0 else ewp[HALF:VW, :]
                else:
                    src = ewp[:]
                nc.gpsimd.dma_gather(
                    out_ap=mb[:].rearrange("p (g d) -> p g d", d=DP)[:, 0:ntc, :],
                    in_ap=src,
                    idxs_ap=msg_ix[:, t0 * 8:(t0 + ntc) * 8],
                    num_idxs=ntc * P, num_idxs_reg=ntc * P,
                    elem_size=DP)
                st = sb_p.tile([P, GT * P], F32R, tag="sb", name="st")
                nc.vector.tensor_tensor(
                    out=st[:].rearrange("p (g q) -> p g q", q=P)[:, 0:ntc, :],
                    in0=scol[:, t0:t0 + ntc].unsqueeze(-1).to_broadcast([P, ntc, P]),
                    in1=iota_f[:].unsqueeze(1).to_broadcast([P, ntc, P]),
                    op=EQ)
                for i in range(ntc):
                    seg_of_tile[t0 + i] = (mb, st, i)
                call_issued[0] += 1

        seg_ranges = {}
        cursor = 0
        for (b, r, h, nt) in stream:
            seg_ranges.setdefault((b, r), []).append((cursor, nt))
            cursor += nt

        for b in range(nb):
            h0b = h0_p.tile([P, D], F32, tag="h0", name="h0b")
            nc.gpsimd.indirect_dma_start(
                out=h0b[:],
                out_offset=None,
                in_=ewp[:].bitcast(F32),
                in_offset=IndirectOffsetOnAxis(ap=h0_idx[:, b:b + 1], axis=0),
            )
            neighs = []
            for r in range(3):
                tiles = [t for (s, n) in seg_ranges[(b, r)]
                         for t in range(s, s + n)]
                ensure_calls(tiles[-1])
                pseg = ps_seg.tile([P, D], F32, space="PSUM", tag="seg", name="pseg")
                for i, t in enumerate(tiles):
                    mb, st, off = seg_of_tile[t]
                    nc.tensor.matmul(
                        out=pseg[:],
                        lhsT=st[:, off * P:(off + 1) * P],
                        rhs=mb[:].rearrange("p (g d) -> p g d", d=DP)[:, off, 0:D],
                        start=(i == 0), stop=(i == len(tiles) - 1),
                    )
                neigh = nei_p.tile([P, D], F32, tag="nei", name="neigh")
                nc.vector.tensor_scalar_mul(
                    out=neigh[:], in0=pseg[:],
                    scalar1=rcnt[:, b * 3 + r:b * 3 + r + 1])
                neighs.append(neigh)

            def transpose_chunks(src):
                outs = []
                for k in range(NCH):
                    pt = ps_tp.tile([DC, P], F32, space="PSUM", tag="tp", name="pt")
                    nc.tensor.transpose(
                        out=pt[:],
                        in_=src[:, k * DC:(k + 1) * DC],
                        identity=ident[:])
                    stx = tp_p.tile([DC, P], F32R, tag="tpo", name="stx")
                    nc.vector.tensor_copy(out=stx[:], in_=pt[:])
                    outs.append(stx)
                return outs

            h0T = transpose_chunks(h0b)
            neighT = [transpose_chunks(neighs[r]) for r in range(3)]

            hs = []
            for r in range(3):
                pmm = ps_mm.tile([P, D], F32, space="PSUM", tag="mm", name="pmm")
                for k in range(NCH):
                    nc.tensor.matmul(
                        out=pmm[:], lhsT=h0T[k][:],
                        rhs=wW[:, (r * 2 * NCH + k) * D:(r * 2 * NCH + k + 1) * D],
                        start=(k == 0), stop=False)
                for k in range(NCH):
                    nc.tensor.matmul(
                        out=pmm[:], lhsT=neighT[r][k][:],
                        rhs=wW[:, ((r * 2 + 1) * NCH + k) * D:((r * 2 + 1) * NCH + k + 1) * D],
                        start=False, stop=False)
                bias_mm(pmm[:], r, P)
                h = h_p.tile([P, D], F32, tag="h", name="h")
                nc.vector.tensor_copy(out=h[:], in_=pmm[:])
                hs.append(h)

            # attention (6 unique gram entries; mult on DVE, row-sum on ACT)
            G = small_p.tile([P, 6], F32, tag="G", name="G")
            pairs = [(0, 0), (0, 1), (0, 2), (1, 1), (1, 2), (2, 2)]
            for i, (a, bb) in enumerate(pairs):
                scr = scr_p.tile([P, D], F32, tag="scr", name="scr")
                nc.vector.tensor_tensor(out=scr[:], in0=hs[a][:], in1=hs[bb][:],
                                        op=MUL)
                scr2 = scr_p.tile([P, D], F32, tag="scrb", name="scr2")
                nc.scalar.activation(out=scr2[:], in_=scr[:], func=Ident,
                                     accum_out=G[:, i:i + 1])
            E = small_p.tile([P, 6], F32, tag="E", name="E")
            nc.scalar.activation(out=E[:], in_=G[:], func=Exp, scale=sqd)
            S3 = small_p.tile([P, 3], F32, tag="S3", name="S3")
            for t_i, (i0, i1, i2) in enumerate([(0, 1, 2), (1, 3, 4), (2, 4, 5)]):
                nc.vector.tensor_tensor(out=S3[:, t_i:t_i + 1],
                                        in0=E[:, i0:i0 + 1], in1=E[:, i1:i1 + 1], op=ADD)
                nc.vector.tensor_tensor(out=S3[:, t_i:t_i + 1],
                                        in0=S3[:, t_i:t_i + 1], in1=E[:, i2:i2 + 1], op=ADD)
            R3 = small_p.tile([P, 3], F32, tag="R3", name="R3")
            nc.vector.tensor_scalar_mul(out=R3[:], in0=S3[:], scalar1=3.0)
            nc.vector.reciprocal(out=R3[:], in_=R3[:])
            AL = small_p.tile([P, 3], F32, tag="AL", name="AL")
            tmp = small_p.tile([P, 1], F32, tag="tmp", name="tmp")
            egrid = [(0, 1, 2), (1, 3, 4), (2, 4, 5)]
            for s_i in range(3):
                i0, i1, i2 = egrid[s_i]
                nc.vector.tensor_tensor(out=AL[:, s_i:s_i + 1],
                                        in0=E[:, i0:i0 + 1], in1=R3[:, 0:1], op=MUL)
                nc.vector.tensor_tensor(out=tmp[:], in0=E[:, i1:i1 + 1],
                                        in1=R3[:, 1:2], op=MUL)
                nc.vector.tensor_tensor(out=AL[:, s_i:s_i + 1],
                                        in0=AL[:, s_i:s_i + 1], in1=tmp[:], op=ADD)
                nc.vector.tensor_tensor(out=tmp[:], in0=E[:, i2:i2 + 1],
                                        in1=R3[:, 2:3], op=MUL)
                nc.vector.tensor_tensor(out=AL[:, s_i:s_i + 1],
                                        in0=AL[:, s_i:s_i + 1], in1=tmp[:], op=ADD)

            if KDBG and b == 0:
                tap = {"h0": h0b, "nei0": neighs[0], "nei1": neighs[1],
                       "hs0": hs[0], "hs1": hs[1]}.get(KDBG)
                if tap is not None:
                    nc.vector.tensor_copy(out=dbg[:], in_=tap[:])
                elif KDBG == "G":
                    nc.vector.tensor_copy(out=dbg[:, 0:6], in_=G[:])
                elif KDBG == "E":
                    nc.vector.tensor_copy(out=dbg[:, 0:6], in_=E[:])
                elif KDBG == "AL":
                    nc.vector.tensor_copy(out=dbg[:, 0:3], in_=AL[:])
            frow = scr_p.tile([P, D], F32, tag="frow", name="frow")
            nc.vector.tensor_scalar_mul(out=frow[:], in0=hs[0][:], scalar1=AL[:, 0:1])
            scr3 = scr_p.tile([P, D], F32, tag="scr", name="scr3")
            for s_i in (1, 2):
                nc.vector.tensor_scalar_mul(out=scr3[:], in0=hs[s_i][:],
                                            scalar1=AL[:, s_i:s_i + 1])
                nc.vector.tensor_tensor(out=frow[:], in0=frow[:], in1=scr3[:], op=ADD)
            nc.vector.tensor_tensor(out=frow[:], in0=frow[:], in1=h0b[:], op=ADD)
            nc.vector.tensor_copy(out=tbl[:, b * D:(b + 1) * D], in_=frow[:])

        # ---- pooling ----
        PHASE = int(os.environ.get("K_PHASE", "9"))
        ndch = (B + P - 1) // P
        if PHASE <= 1:
            ltmp = scr_p.tile([bl, C], F32, tag="lg", name="ltmp")
            nc.vector.memset(ltmp[:], 0.0)
            nc.sync.dma_start(out=out_logits[:], in_=ltmp[:])
            dtmp = scr_p.tile([bl, D], F32, tag="dres", name="dtmp")
            if KDBG and KDBG != "tbl":
                nc.vector.tensor_copy(out=dtmp[:], in_=dbg[:bl, :])
            else:
                nc.vector.tensor_copy(out=dtmp[:], in_=tbl[:bl, 0:D])
            nc.sync.dma_start(out=out_docrep[:], in_=dtmp[:])
            do_rest = False
        else:
            do_rest = True

        if do_rest:
            tpart = scr_p.tile([P, ndch * D], F32, tag="tpart", name="tpart")
            for j in range(ndch):
                cw = min(P, B - j * P)
                pj = ps_mm.tile([cw, D], F32, space="PSUM", tag="mm", name="pj")
                for b in range(nb):
                    cmt = cm_p.tile([P, cw], F32R, tag="cm", name="cmt")
                    nc.gpsimd.dma_start(
                        out=cmt[:],
                        in_=cmat_d[:].rearrange("p (n q) -> p n q", q=B)[:, b, j * P:j * P + cw])
                    nc.tensor.matmul(out=pj[:], lhsT=cmt[:],
                                     rhs=tbl[:, b * D:(b + 1) * D],
                                     start=(b == 0), stop=(b == nb - 1))
                nc.vector.tensor_scalar_mul(out=tpart[:cw, j * D:(j + 1) * D],
                                            in0=pj[:], scalar1=1.0 / L)

            tp_dram = dram_p.tile([B, D], F32, name="tp_dram")
            for j in range(ndch):
                rows = min(P, B - j * P)
                nc.sync.dma_start(out=tp_dram[j * P:j * P + rows, :],
                                  in_=tpart[:rows, j * D:(j + 1) * D])
            rs_dram = dram_p.tile([bl, D], F32, name="rs_dram")
            if os.environ.get("K_SKIP_CC") == "1":
                nc.sync.dma_start(out=rs_dram[:], in_=tp_dram[:bl, :])
            else:
                nc.gpsimd.collective_compute(
                    "ReduceScatter", ADD,
                    replica_groups=[list(range(NCORES))],
                    ins=[tp_dram[:]], outs=[rs_dram[:]],
                )
            tmean = scr_p.tile([bl, D], F32, tag="tmean", name="tmean")
            nc.sync.dma_start(out=tmean[:], in_=rs_dram[:])

        if do_rest and PHASE <= 2:
            ltmp = scr_p.tile([bl, C], F32, tag="lg", name="ltmp2")
            nc.vector.memset(ltmp[:], 0.0)
            nc.sync.dma_start(out=out_logits[:], in_=ltmp[:])
            nc.sync.dma_start(out=out_docrep[:], in_=tmean[:])
            do_rest = False

        # ---- doc phase ----
        if do_rest:
            wDocT = w_p.tile([DC, 6 * NCH * D], F32R, tag="wbig2", name="wDocT")
            nc.gpsimd.dma_start(out=wDocT[:], in_=wDoc_d[:])

            def transpose_small(src):
                outs = []
                for k in range(NCH):
                    pt = ps_tp.tile([DC, bl], F32, space="PSUM", tag="tp", name="pts")
                    nc.tensor.transpose(
                        out=pt[:],
                        in_=src[:, k * DC:(k + 1) * DC],
                        identity=identS[:])
                    stx = tp_p.tile([DC, bl], F32R, tag="tpos", name="stxs")
                    nc.vector.tensor_copy(out=stx[:], in_=pt[:])
                    outs.append(stx)
                return outs

            tmT = transpose_small(tmean)
            pdo = ps_mm.tile([bl, D], F32, space="PSUM", tag="mm", name="pdo")
            for k in range(NCH):
                nc.tensor.matmul(out=pdo[:], lhsT=tmT[k][:],
                                 rhs=wdense[:, k * D:(k + 1) * D],
                                 start=(k == 0), stop=False)
            bias_mm(pdo[:], 6, bl)
            doc_out = scr_p.tile([bl, D], F32, tag="doc_out", name="doc_out")
            nc.vector.tensor_copy(out=doc_out[:], in_=pdo[:])
            doT = transpose_small(doc_out)

            # doc-graph neighbor means (dma_gather from padded emb_doc)
            ZGT = 2
            dneighs = []
            for r in range(3):
                pn = ps_seg.tile([bl, D], F32, space="PSUM", tag="seg", name="pn")
                nchunk = (nzc + ZGT - 1) // ZGT
                for cchunk in range(nchunk):
                    q0 = cchunk * ZGT
                    nq = min(ZGT, nzc - q0)
                    zt = zd_p.tile([P, ZGT * DP], F32R, tag="zd", name="zt")
                    nc.gpsimd.dma_gather(
                        out_ap=zt[:].rearrange("p (g d) -> p g d", d=DP)[:, 0:nq, :],
                        in_ap=edp[:],
                        idxs_ap=zidx[:, (r * nzc + q0) * 8:(r * nzc + q0 + nq) * 8],
                        num_idxs=nq * P, num_idxs_reg=nq * P,
                        elem_size=DP)
                    for qq in range(nq):
                        q = q0 + qq
                        nc.tensor.matmul(
                            out=pn[:],
                            lhsT=cdm[:, (r * nzc + q) * bl:(r * nzc + q + 1) * bl],
                            rhs=zt[:].rearrange("p (g d) -> p g d", d=DP)[:, qq, 0:D],
                            start=(q == 0), stop=(q == nzc - 1))
                dnei = nei_p.tile([bl, D], F32, tag="dnei", name="dnei")
                nc.vector.tensor_scalar_mul(out=dnei[:], in0=pn[:],
                                            scalar1=rcd[:, r:r + 1])
                dneighs.append(dnei)

            dhs = []
            for r in range(3):
                dnT = transpose_small(dneighs[r])
                pm = ps_mm.tile([bl, D], F32, space="PSUM", tag="mm", name="pm")
                for k in range(NCH):
                    nc.tensor.matmul(
                        out=pm[:], lhsT=doT[k][:],
                        rhs=wDocT[:, (r * 2 * NCH + k) * D:(r * 2 * NCH + k + 1) * D],
                        start=(k == 0), stop=False)
                for k in range(NCH):
                    nc.tensor.matmul(
                        out=pm[:], lhsT=dnT[k][:],
                        rhs=wDocT[:, ((r * 2 + 1) * NCH + k) * D:((r * 2 + 1) * NCH + k + 1) * D],
                        start=False, stop=False)
                bias_mm(pm[:], 3 + r, bl)
                dh = h_p.tile([bl, D], F32, tag="dh", name="dh")
                nc.vector.tensor_copy(out=dh[:], in_=pm[:])
                dhs.append(dh)

            Gd = small_p.tile([bl, 6], F32, tag="Gd", name="Gd")
            pairs = [(0, 0), (0, 1), (0, 2), (1, 1), (1, 2), (2, 2)]
            for i, (a, bb) in enumerate(pairs):
                scrd = scr_p.tile([bl, D], F32, tag="scrd", name="scrd")
                nc.vector.tensor_tensor(out=scrd[:], in0=dhs[a][:], in1=dhs[bb][:],
                                        op=MUL)
                scrd2 = scr_p.tile([bl, D], F32, tag="scrd2", name="scrd2")
                nc.scalar.activation(out=scrd2[:], in_=scrd[:], func=Ident,
                                     accum_out=Gd[:, i:i + 1])
            Ed = small_p.tile([bl, 6], F32, tag="Ed", name="Ed")
            nc.scalar.activation(out=Ed[:], in_=Gd[:], func=Exp, scale=sqd)
            S3d = small_p.tile([bl, 3], F32, tag="S3d", name="S3d")
            for t_i, (i0, i1, i2) in enumerate([(0, 1, 2), (1, 3, 4), (2, 4, 5)]):
                nc.vector.tensor_tensor(out=S3d[:, t_i:t_i + 1],
                                        in0=Ed[:, i0:i0 + 1], in1=Ed[:, i1:i1 + 1], op=ADD)
                nc.vector.tensor_tensor(out=S3d[:, t_i:t_i + 1],
                                        in0=S3d[:, t_i:t_i + 1], in1=Ed[:, i2:i2 + 1], op=ADD)
            R3d = small_p.tile([bl, 3], F32, tag="R3d", name="R3d")
            nc.vector.tensor_scalar_mul(out=R3d[:], in0=S3d[:], scalar1=3.0)
            nc.vector.reciprocal(out=R3d[:], in_=R3d[:])
            ALd = small_p.tile([bl, 3], F32, tag="ALd", name="ALd")
            tmpd = small_p.tile([bl, 1], F32, tag="tmpd", name="tmpd")
            egrid = [(0, 1, 2), (1, 3, 4), (2, 4, 5)]
            for s_i in range(3):
                i0, i1, i2 = egrid[s_i]
                nc.vector.tensor_tensor(out=ALd[:, s_i:s_i + 1],
                                        in0=Ed[:, i0:i0 + 1], in1=R3d[:, 0:1], op=MUL)
                nc.vector.tensor_tensor(out=tmpd[:], in0=Ed[:, i1:i1 + 1],
                                        in1=R3d[:, 1:2], op=MUL)
                nc.vector.tensor_tensor(out=ALd[:, s_i:s_i + 1],
                                        in0=ALd[:, s_i:s_i + 1], in1=tmpd[:], op=ADD)
                nc.vector.tensor_tensor(out=tmpd[:], in0=Ed[:, i2:i2 + 1],
                                        in1=R3d[:, 2:3], op=MUL)
                nc.vector.tensor_tensor(out=ALd[:, s_i:s_i + 1],
                                        in0=ALd[:, s_i:s_i + 1], in1=tmpd[:], op=ADD)

            dres = scr_p.tile([bl, D], F32, tag="dres", name="dres")
            nc.vector.tensor_scalar_mul(out=dres[:], in0=dhs[0][:], scalar1=ALd[:, 0:1])
            scrd3 = scr_p.tile([bl, D], F32, tag="scrd", name="scrd3")
            for s_i in (1, 2):
                nc.vector.tensor_scalar_mul(out=scrd3[:], in0=dhs[s_i][:],
                                            scalar1=ALd[:, s_i:s_i + 1])
                nc.vector.tensor_tensor(out=dres[:], in0=dres[:], in1=scrd3[:], op=ADD)
            nc.vector.tensor_tensor(out=dres[:], in0=dres[:], in1=doc_out[:], op=ADD)
            nc.sync.dma_start(out=out_docrep[:], in_=dres[:])

            drT = transpose_small(dres)
            pfc = ps_mm.tile([bl, C], F32, space="PSUM", tag="mm", name="pfc")
            for k in range(NCH):
                nc.tensor.matmul(out=pfc[:], lhsT=drT[k][:],
                                 rhs=wfc[:, k * C:(k + 1) * C],
                                 start=(k == 0), stop=False)
            bias_mm(pfc[:], 7, bl, width=C)
            lg = scr_p.tile([bl, C], F32, tag="lg", name="lg")
            nc.vector.tensor_copy(out=lg[:], in_=pfc[:])
            nc.sync.dma_start(out=out_logits[:], in_=lg[:])

    nc.compile()
    return nc


# --------------------------------------------------------------------------
# entry point
# --------------------------------------------------------------------------

def kernel(**inputs):
    inputs = {k: np.asarray(v) for k, v in inputs.items()}
    meta, in_maps = prep(inputs)
    nc = build_program(meta)
    res = run_bass_kernel_spmd(nc, in_maps, core_ids=list(range(NCORES)))
    want_docrep = int(np.asarray(inputs.get("return_doc_representation", 0)))
    key = "out_docrep" if want_docrep else "out_logits"
    out = np.concatenate([res.results[c][key] for c in range(NCORES)], axis=0)
    return out.astype(np.float32)

